# revision 12
# baseline (speedup 1.0000x reference)
"""Trainium2 Bass kernel for nn_ExampleGuidedAttention.

Per-sample computation (N=8 samples, one per NeuronCore):
    Q   = F @ Wq                      # (4096, 64),  F = src_feature (4096, 256)
    S   = Q @ Q^T                     # (4096, 4096), symmetric
    A   = softmax_rows(S)             # softmax over axis -1
    att = A^T @ F                     # (4096, 256)  [the module applies the map transposed]
    out = concat(m * att + (1-m) * ref, att)   # (4096, 512)

The softmax normalizer runs along the *contraction* axis of the apply
matmul, so a flash-style single pass is impossible.  Two passes:
  pass 1: l_p = sum_q exp(S[p, q])   (exp + row-sum fused on ScalarE)
  pass 2: out[q, :] += sum_p exp(S[p, q]) * (F[p, :] / l_p)
exp() needs no max-subtraction: S values are bounded (|S| <~ 12) by
construction, far inside fp32 exp range.

Matmuls run in bf16 (fp32 PSUM accumulation).  E-cache: the first
E_CACHE_BLOCKS p-blocks of exp(S) (bf16) are kept in SBUF from pass 1 so
pass 2 skips their S-matmul + exp entirely.
"""

import numpy as np

import concourse.bass as bass
import concourse.mybir as mybir
import concourse.tile as tile
from concourse.bass_utils import run_bass_kernel_spmd
from concourse.masks import make_identity

# ---------------------------------------------------------------------------
# Patch: the walrus build in this container rejects instructions carrying
# more than 2 sync waits (CoreV3 setupSyncWait "Too many sync wait
# commands").  TileContext's final drain carries one wait per live logical
# processor; split them across preceding sync-queue nops, one wait each.
_orig_drain_and_barrier = tile.TileContext._drain_and_barrier


def _patched_drain_and_barrier(self, tick_clock, wait_clock):
    import bass_rust as _br

    nc = self.nc
    spare_nops = [nc.sync.nop(nofuse=True) for _ in range(32)]
    drain_inst = nc.sync.drain()
    wait_clock.add_sem_waits(
        drain_inst.ins, tile.ScopedClock({None: tick_clock.global_clock})
    )
    waits = list(drain_inst.ins.sync_info.on_wait)
    if len(waits) > 1:
        assert len(waits) <= len(spare_nops) + 1, len(waits)
        drain_inst.ins.sync_info.on_wait = [waits[-1]]
        for nop, w in zip(spare_nops, waits[:-1]):
            nop.ins.sync_info = _br.SyncInfo(on_wait=[w], on_update=[])

    nc.all_engine_barrier()
    assert self.sems is not None
    popped = nc._tile_sem_poison_stack.pop()
    assert popped is self._sem_poison
    nc.clear_and_free_semaphores(list(self.sems.allocated().values()))
    nc.all_engine_barrier()


tile.TileContext._drain_and_barrier = _patched_drain_and_barrier

_MAX_WAITS = 1


def _split_sync_waits(nc, max_waits=_MAX_WAITS):
    """Same walrus limitation as above, applied to every instruction: hoist
    excess sync waits onto preceding nops on the same engine queue (queues
    are FIFO, so the waits still gate the original instruction)."""
    import bass_rust as _br

    ctr = 0
    for fn in nc.m.functions:
        for bb in fn.blocks:
            insts = bb.instructions
            if not any(
                i.sync_info is not None and len(i.sync_info.on_wait) > max_waits
                for i in insts
            ):
                continue
            new = []
            for inst in insts:
                si = inst.sync_info
                if si is not None and len(si.on_wait) > max_waits:
                    waits = list(si.on_wait)
                    extra, keep = waits[:-max_waits], waits[-max_waits:]
                    for g0 in range(0, len(extra), max_waits):
                        grp = extra[g0:g0 + max_waits]
                        nop = _br.InstNoOp(name=f"I-waitsplit-{ctr}")
                        ctr += 1
                        nop.engine = inst.engine
                        nop.sync_info = _br.SyncInfo(on_wait=grp, on_update=[])
                        new.append(nop)
                    si.on_wait = keep
                new.append(inst)
            bb.instructions = new
# ---------------------------------------------------------------------------

N, H, W, C = 8, 64, 64, 256
P = H * W            # 4096 pixels
D = C // 4           # 64 query channels
NB = P // 128        # 32 p-blocks of 128 rows
NPAIR = NB // 2      # 16 row-packed S-matmul pairs
RQ = 512             # q-columns per pass-2 round
NR = P // RQ         # 8 rounds
E_CACHE_BLOCKS = 0   # p-blocks of exp(S) kept resident in SBUF (even number)

F32 = mybir.dt.float32
BF16 = mybir.dt.bfloat16
EXP = mybir.ActivationFunctionType.Exp
SUB = mybir.AluOpType.subtract


def build_nc(e_cache_blocks=E_CACHE_BLOCKS, split_waits=True):
    assert e_cache_blocks % 2 == 0
    cached_pairs = e_cache_blocks // 2

    nc = bass.Bass()
    mask_d = nc.dram_tensor("src_mask", [P], F32, kind="ExternalInput")
    feat_d = nc.dram_tensor("src_feature", [P, C], F32, kind="ExternalInput")
    ref_d = nc.dram_tensor("ref_feature", [P, C], F32, kind="ExternalInput")
    wq_d = nc.dram_tensor("w_query", [C, D], F32, kind="ExternalInput")
    out_d = nc.dram_tensor("out", [P, 2 * C], F32, kind="ExternalOutput")

    feat_r = feat_d.rearrange("(b p) c -> p b c", p=128)   # [128, NB, C]
    wq_r = wq_d.rearrange("(a p) d -> p a d", p=128)       # [128, 2, D]
    # The reference reshapes the channel-major (C, HW) attention output
    # directly to (H, W, C) — a flat memory reinterpretation.  In AO
    # coordinates [ci, q] (q = 256*b + c2, b = x % 16, x = 16*ci + b):
    #   out[x, 256+c2] = AO[ci, q]
    #   out[x, c2]     = m[x] * AO[ci, q] + (1 - m[x]) * ref[x, c2]
    # and ref/mask flat-viewed match: ref_v[ci, q] = ref[16ci + b, c2].
    out_v = out_d.rearrange("(ci b) (h c) -> ci b h c", b=16, h=2)
    ref_v = ref_d.rearrange("(a b) c -> a (b c)", b=16)    # [256, 4096]
    mask_v = mask_d.rearrange("(ci b) -> ci b", b=16)      # [256, 16]

    with tile.TileContext(nc) as tc:
        with (
            tc.tile_pool(name="persist", bufs=1) as persist,
            tc.tile_pool(name="fstage", bufs=2) as fstage,
            tc.tile_pool(name="e1pool", bufs=3) as e1pool,
            tc.tile_pool(name="e2pool", bufs=3) as e2pool,
            tc.tile_pool(name="refpool", bufs=2) as refpool,
            tc.tile_pool(name="opool", bufs=2) as opool,
            tc.tile_pool(name="btmp", bufs=4) as btmp,
        ):
            # ------------------------------------------------ persistent tiles
            f_bf = persist.tile([128, NB, C], BF16)        # F then F' (bf16)
            ft_bf = persist.tile([128, 2, P], BF16)        # F^T (c on partitions)
            qt2 = persist.tile([128, P], BF16)             # Q^T duplicated on both
            #                                                partition halves
            w_sb = persist.tile([128, 2, D], F32)
            w_bf = persist.tile([128, 2, D], BF16)
            ident = persist.tile([128, 128], BF16)
            m_sb = persist.tile([128, 2, 16], F32)     # mask in AO coords [ci, b]
            l_parts = persist.tile([128, NB, 2], F32)
            l_sb = persist.tile([128, NB], F32)
            linv = persist.tile([128, NB], F32)
            if cached_pairs:
                e_cache = persist.tile([128, 2 * cached_pairs, P], BF16)

            # ------------------------------------------------ phase 0: loads
            nc.sync.dma_start(out=w_sb, in_=wq_r)
            nc.vector.tensor_copy(out=w_bf, in_=w_sb)
            for cc in range(2):
                nc.sync.dma_start(
                    out=m_sb[:, cc, :],
                    in_=mask_v[cc * 128:(cc + 1) * 128, :],
                )
            make_identity(nc, ident)

            # F: load fp32 (streamed), cast to bf16
            for i in range(4):
                fs = fstage.tile([128, NB // 4, C], F32)
                sl = slice(i * (NB // 4), (i + 1) * (NB // 4))
                nc.sync.dma_start(out=fs, in_=feat_r[:, sl, :])
                nc.vector.tensor_copy(out=f_bf[:, sl, :], in_=fs)

            with tc.tile_pool(name="ptp", bufs=2, space="PSUM") as ptp, \
                 tc.tile_pool(name="pqt", bufs=2, space="PSUM") as pqt:
                # F^T via PE transposes: 64 x [128,128], 8 per PSUM tile
                for j in range(2):
                    for bg in range(4):
                        tp = ptp.tile([128, 8, 128], BF16)
                        for k in range(8):
                            b = bg * 8 + k
                            nc.tensor.transpose(
                                tp[:, k, :],
                                f_bf[:, b, j * 128:(j + 1) * 128],
                                ident,
                            )
                        nc.scalar.copy(
                            out=ft_bf[:, j, bg * 1024:(bg + 1) * 1024],
                            in_=tp,
                        )
                # Q^T = (Wq^T @ F^T) : [64, P] in chunks of 512
                for qc in range(8):
                    qs = slice(qc * 512, (qc + 1) * 512)
                    qt_ps = pqt.tile([64, 512], F32)
                    nc.tensor.matmul(
                        qt_ps, lhsT=w_bf[:, 0, :], rhs=ft_bf[:, 0, qs],
                        start=True, stop=False,
                    )
                    nc.tensor.matmul(
                        qt_ps, lhsT=w_bf[:, 1, :], rhs=ft_bf[:, 1, qs],
                        start=False, stop=True,
                    )
                    nc.vector.tensor_copy(out=qt2[0:64, qs], in_=qt_ps)
            # duplicate Q^T onto partitions 64..127 (for row-packed matmuls)
            nc.sync.dma_start(out=qt2[64:128, :], in_=qt2[0:64, :])

            # ------------------------------------------------ phase 1: l_p
            with tc.tile_pool(name="ps1", bufs=2, space="PSUM") as ps1:
                for pair in range(NPAIR):
                    b0, b1 = 2 * pair, 2 * pair + 1
                    for h in range(2):      # q halves of 2048
                        sA = ps1.tile([128, 2048], F32, tag="s1")
                        sB = ps1.tile([128, 2048], F32, tag="s1")
                        for qc in range(4):
                            cs = slice(qc * 512, (qc + 1) * 512)
                            gs = slice(h * 2048 + qc * 512,
                                       h * 2048 + (qc + 1) * 512)
                            nc.tensor.matmul(
                                sA[:, cs],
                                lhsT=qt2[0:64, b0 * 128:(b0 + 1) * 128],
                                rhs=qt2[0:64, gs],
                                start=True, stop=True,
                            )
                            nc.tensor.matmul(
                                sB[:, cs],
                                lhsT=qt2[64:128, b1 * 128:(b1 + 1) * 128],
                                rhs=qt2[64:128, gs],
                                start=True, stop=True,
                            )
                        for b, s_ps in ((b0, sA), (b1, sB)):
                            if pair < cached_pairs:
                                edst = e_cache[:, b, h * 2048:(h + 1) * 2048]
                            else:
                                edst = e1pool.tile([128, 2048], BF16)
                            nc.scalar.activation(
                                out=edst, in_=s_ps, func=EXP,
                                accum_out=l_parts[:, b, h:h + 1],
                            )
                # l, 1/l, F' = F/l
                nc.vector.reduce_sum(l_sb, l_parts, axis=mybir.AxisListType.X)
                nc.vector.reciprocal(linv, l_sb)
                for b in range(NB):
                    nc.vector.tensor_scalar_mul(
                        out=f_bf[:, b, :], in0=f_bf[:, b, :],
                        scalar1=linv[:, b:b + 1],
                    )

            # ------------------------------------------------ phase 2: apply
            # AO[ci, q] += sum_p F'[p, ci] * exp(S[p, q]) : F' stationary,
            # E moving, output channel-major (as the reference reshapes it).
            with tc.tile_pool(name="pao", bufs=4, space="PSUM") as pao, \
                 tc.tile_pool(name="ps2", bufs=2, space="PSUM") as ps2:
                for r in range(NR):
                    rs = slice(r * RQ, (r + 1) * RQ)
                    ref_t = refpool.tile([128, 2, RQ], F32)
                    for cc in range(2):
                        nc.sync.dma_start(
                            out=ref_t[:, cc, :],
                            in_=ref_v[cc * 128:(cc + 1) * 128, rs],
                        )
                    ao_ps = [pao.tile([128, RQ], F32, tag="ao",
                                      name=f"ao_ps_{r}_{t}")
                             for t in range(2)]
                    for pair in range(NPAIR):
                        b0, b1 = 2 * pair, 2 * pair + 1
                        if pair < cached_pairs:
                            e_rhs = e_cache[:, 2 * pair:2 * pair + 2, rs]
                        else:
                            s2 = ps2.tile([128, 2, RQ], F32, tag="s2")
                            nc.tensor.matmul(
                                s2[:, 0, :],
                                lhsT=qt2[0:64, b0 * 128:(b0 + 1) * 128],
                                rhs=qt2[0:64, rs],
                                start=True, stop=True,
                            )
                            nc.tensor.matmul(
                                s2[:, 1, :],
                                lhsT=qt2[64:128, b1 * 128:(b1 + 1) * 128],
                                rhs=qt2[64:128, rs],
                                start=True, stop=True,
                            )
                            e2 = e2pool.tile([128, 2, RQ], BF16)
                            nc.scalar.activation(out=e2, in_=s2, func=EXP)
                            e_rhs = e2
                        for cc in range(2):
                            cs = slice(cc * 128, (cc + 1) * 128)
                            nc.tensor.matmul(
                                ao_ps[cc], lhsT=f_bf[:, b0, cs],
                                rhs=e_rhs[:, 0, :],
                                start=(pair == 0), stop=False,
                            )
                            nc.tensor.matmul(
                                ao_ps[cc], lhsT=f_bf[:, b1, cs],
                                rhs=e_rhs[:, 1, :],
                                start=False, stop=(pair == NPAIR - 1),
                            )
                    # epilogue: att copy + mask blend in AO coords, store
                    for cc in range(2):
                        ostage = opool.tile([128, 2, 2, C], F32,
                                            tag="ostage",
                                            name=f"ostage_{r}_{cc}")
                        ao_v = ao_ps[cc].rearrange("p (b c) -> p b c", b=2)
                        nc.scalar.copy(out=ostage[:, :, 1, :], in_=ao_v)
                        tmp = btmp.tile([128, RQ], F32)
                        nc.vector.tensor_sub(
                            out=tmp, in0=ao_ps[cc], in1=ref_t[:, cc, :],
                        )
                        tmp_v = tmp.rearrange("p (b c) -> p b c", b=2)
                        nc.vector.tensor_mul(
                            out=tmp_v, in0=tmp_v,
                            in1=m_sb[:, cc, 2 * r:2 * r + 2].broadcast_to(
                                (128, 2, C)),
                        )
                        nc.vector.tensor_add(
                            out=ostage[:, :, 0, :], in0=tmp_v,
                            in1=ref_t[:, cc, :].rearrange(
                                "p (b c) -> p b c", b=2),
                        )
                        nc.sync.dma_start(
                            out=out_v[cc * 128:(cc + 1) * 128,
                                      2 * r:2 * r + 2, :, :],
                            in_=ostage,
                        )
    if split_waits:
        _split_sync_waits(nc)
    return nc


_nc_cache = {}


def _get_nc(e_cache_blocks=E_CACHE_BLOCKS):
    if e_cache_blocks not in _nc_cache:
        _nc_cache[e_cache_blocks] = build_nc(e_cache_blocks)
    return _nc_cache[e_cache_blocks]


def run(inputs, trace=False, e_cache_blocks=E_CACHE_BLOCKS):
    nc = _get_nc(e_cache_blocks)
    in_maps = []
    for n in range(N):
        in_maps.append({
            "src_mask": np.ascontiguousarray(
                np.asarray(inputs["src_mask"][n], dtype=np.float32).reshape(P)),
            "src_feature": np.ascontiguousarray(
                np.asarray(inputs["src_feature"][n], dtype=np.float32
                           ).reshape(P, C)),
            "ref_feature": np.ascontiguousarray(
                np.asarray(inputs["ref_feature"][n], dtype=np.float32
                           ).reshape(P, C)),
            "w_query": np.ascontiguousarray(
                np.asarray(inputs["w_query"], dtype=np.float32)),
        })
    res = run_bass_kernel_spmd(nc, in_maps, core_ids=list(range(N)),
                               trace=trace)
    out = np.stack([np.asarray(r["out"]) for r in res.results])
    return out.reshape(N, H, W, 2 * C), res


def kernel(src_mask, src_feature, ref_feature, w_query):
    out, _ = run({
        "src_mask": src_mask, "src_feature": src_feature,
        "ref_feature": ref_feature, "w_query": w_query,
    })
    return out


def make_bench(nc, in_maps):
    """Build a jitted runner with device-resident inputs and no donation so
    repeated calls do zero host transfers; per-call wall time over a
    pipelined batch approximates HW exec time."""
    import jax
    import concourse.mybir as mb
    from concourse import bass2jax
    from jax.experimental.shard_map import shard_map
    from jax.sharding import Mesh, NamedSharding, PartitionSpec

    bass2jax.install_neuronx_cc_hook()
    n_cores = len(in_maps)
    part_name = (nc.partition_id_tensor.name
                 if nc.partition_id_tensor else None)
    in_names, out_names, out_avals, zero_outs = [], [], [], []
    for alloc in nc.m.functions[0].allocations:
        if not isinstance(alloc, mb.MemoryLocationSet):
            continue
        name = alloc.memorylocations[0].name
        if alloc.kind == "ExternalInput":
            if name != part_name:
                in_names.append(name)
        elif alloc.kind == "ExternalOutput":
            out_names.append(name)
            shape = tuple(alloc.tensor_shape)
            dtype = mb.dt.np(alloc.dtype)
            out_avals.append(jax.core.ShapedArray(shape, dtype))
            zero_outs.append(np.zeros(shape, dtype))
    n_params = len(in_names)
    all_in_names = in_names + out_names
    if part_name is not None:
        all_in_names = all_in_names + [part_name]

    def _body(*args):
        operands = list(args)
        if part_name is not None:
            operands.append(bass2jax.partition_id_tensor())
        outs = bass2jax._bass_exec_p.bind(
            *operands,
            out_avals=tuple(out_avals),
            in_names=tuple(all_in_names),
            out_names=tuple(out_names),
            lowering_input_output_aliases=(),
            sim_require_finite=True,
            sim_require_nnan=True,
            nc=nc,
        )
        return tuple(outs)

    devices = jax.devices()[:n_cores]
    mesh = Mesh(np.asarray(devices), ("core",))
    nin = n_params + len(out_names)
    sharded = jax.jit(
        shard_map(
            _body, mesh=mesh,
            in_specs=(PartitionSpec("core"),) * nin,
            out_specs=(PartitionSpec("core"),) * len(out_names),
            check_rep=False,
        ),
        keep_unused=True,
    )
    sh = NamedSharding(mesh, PartitionSpec("core"))
    concat_in = [
        np.concatenate([np.asarray(m[nm]) for m in in_maps], axis=0)
        for nm in in_names
    ] + [
        np.zeros((n_cores * z.shape[0], *z.shape[1:]), z.dtype)
        for z in zero_outs
    ]
    dev_in = [jax.device_put(a, sh) for a in concat_in]

    def bench(k=16):
        import time
        outs = sharded(*dev_in)
        jax.block_until_ready(outs)
        t0 = time.perf_counter()
        for _ in range(k):
            outs = sharded(*dev_in)
        jax.block_until_ready(outs)
        t1 = time.perf_counter()
        return (t1 - t0) / k

    return bench
